# revision 7
# baseline (speedup 1.0000x reference)
"""Trainium2 Bass kernel for gated sparse attention (nn_Attention_1915555414563).

Strategy: data-parallel over batch across 8 cores (8 batches/core).
Per-core pipeline keeps scores TRANSPOSED (S[j,i]: key j on partitions,
query i free) so attn@v needs no on-device transpose of the probability
matrix:
  - host pre-scales Wq by DH**-0.5, splits Wkv, and ships exp(bias)^T
    (bf16) so the additive attention bias becomes one multiply that can
    ride the bf16 2x vector mode.
  - key-side mask folds into the Exp activation's per-partition bias.
  - an all-ones column appended to V yields the softmax denominators as
    row 64 of the attn@v PSUM tile (no separate reduction).
  - fully-masked queries are fixed up afterwards with a predicated copy
    of mean(v) (= softmax of an all-equal row), matching the reference.
"""

import numpy as np
import ml_dtypes

import concourse.bass as bass
import concourse.bacc as bacc
import concourse.tile as tile
from concourse import mybir
from concourse.bass_utils import run_bass_kernel_spmd
from concourse.masks import make_identity

B, N, DIM = 64, 512, 256
H, DH = 8, 64
INNER = H * DH
SCALE = DH ** -0.5
NCORES = 8
BPC = B // NCORES  # batches per core

F32 = mybir.dt.float32
F32R = mybir.dt.float32r
BF16 = mybir.dt.bfloat16

# dtype used for tensor-engine operands that are stored as fp32
MM = F32R

P = 128  # partitions
CC = DIM // P    # 2 contraction chunks of 128
ET = INNER // P  # 4 chunks over the inner (head*dh) dim
IT = N // P      # 4 tiles over the sequence dim
NEG = -60000.0   # exp(x + NEG) == 0 for any realistic score


def build_kernel():
    nc = bacc.Bacc()

    x = nc.dram_tensor("x", [BPC, N, DIM], F32, kind="ExternalInput")
    mjb = nc.dram_tensor("mjb", [BPC, N], F32, kind="ExternalInput")
    pred = nc.dram_tensor("pred", [BPC, N], mybir.dt.uint8, kind="ExternalInput")
    expb = nc.dram_tensor("expb", [H, N, N], BF16, kind="ExternalInput")
    vmt = nc.dram_tensor("vmt", [BPC, INNER], F32, kind="ExternalInput")
    onesd = nc.dram_tensor("onesd", [1, DH], F32R, kind="ExternalInput")
    wq = nc.dram_tensor("wq", [DIM, INNER], F32R, kind="ExternalInput")
    wk = nc.dram_tensor("wk", [DIM, INNER], F32R, kind="ExternalInput")
    wv = nc.dram_tensor("wv", [DIM, INNER], F32R, kind="ExternalInput")
    wg = nc.dram_tensor("wg", [DIM, INNER], F32R, kind="ExternalInput")
    wo = nc.dram_tensor("wo", [INNER, DIM], F32R, kind="ExternalInput")
    bg = nc.dram_tensor("bg", [INNER], F32, kind="ExternalInput")
    bo = nc.dram_tensor("bo", [DIM], F32, kind="ExternalInput")
    out = nc.dram_tensor("out", [BPC, N, DIM], F32, kind="ExternalOutput")

    with tile.TileContext(nc) as tc:
        with (
            tc.tile_pool(name="consts", bufs=1) as consts,
            tc.tile_pool(name="batch", bufs=2) as bp,
            tc.tile_pool(name="head", bufs=2) as hp,
            tc.tile_pool(name="ps_proj", bufs=2, space="PSUM") as ps_proj,
            tc.tile_pool(name="ps_s", bufs=2, space="PSUM") as ps_sp,
            tc.tile_pool(name="ps_ot", bufs=2, space="PSUM") as ps_otp,
            tc.tile_pool(name="ps_rb", bufs=2, space="PSUM") as ps_rbp,
        ):
            # ---- constants (loaded once per core) ----
            wq_t = consts.tile([P, CC, INNER], F32R, tag="wq")
            for _t in range(CC):
                nc.sync.dma_start(out=wq_t[:, _t, :], in_=wq[_t * P:(_t + 1) * P, :])
            wk_t = consts.tile([P, CC, INNER], F32R, tag="wk")
            for _t in range(CC):
                nc.sync.dma_start(out=wk_t[:, _t, :], in_=wk[_t * P:(_t + 1) * P, :])
            wv_t = consts.tile([P, CC, INNER], F32R, tag="wv")
            for _t in range(CC):
                nc.sync.dma_start(out=wv_t[:, _t, :], in_=wv[_t * P:(_t + 1) * P, :])
            wg_t = consts.tile([P, CC, INNER], F32R, tag="wg")
            for _t in range(CC):
                nc.sync.dma_start(out=wg_t[:, _t, :], in_=wg[_t * P:(_t + 1) * P, :])
            wo_t = consts.tile([P, ET, DIM], F32R, tag="wo")
            for _t in range(ET):
                nc.sync.dma_start(out=wo_t[:, _t, :], in_=wo[_t * P:(_t + 1) * P, :])
            bg_t = consts.tile([P, ET], F32, tag="bg")
            nc.sync.dma_start(out=bg_t, in_=bg[:].rearrange("(t p) -> p t", p=P))
            bo_t = consts.tile([P, DIM], F32, tag="bo")
            bo_b = bass.AP(tensor=bo[:].tensor, offset=bo[:].offset,
                           ap=[[0, P]] + bo[:].ap)
            nc.sync.dma_start(out=bo_t, in_=bo_b)
            expb_t = consts.tile([P, H, IT, N], BF16, tag="expb")
            for _h in range(H):
                for _jt in range(IT):
                    nc.sync.dma_start(out=expb_t[:, _h, _jt, :],
                                      in_=expb[_h, _jt * P:(_jt + 1) * P, :])
            ident = consts.tile([P, P], F32, tag="ident")
            make_identity(nc, ident)
            ones1 = consts.tile([1, DH], F32R, tag="ones1")
            nc.sync.dma_start(out=ones1, in_=onesd[:])

            for b in range(BPC):
                # ---- load x, masks ----
                x_t = bp.tile([P, IT, DIM], F32, tag="x")
                for _it in range(IT):
                    nc.sync.dma_start(out=x_t[:, _it, :],
                                      in_=x[b, _it * P:(_it + 1) * P, :])
                mjb_t = bp.tile([P, IT], F32, tag="mjb")
                nc.sync.dma_start(
                    out=mjb_t, in_=mjb[b].rearrange("(jt p) -> p jt", p=P))
                pred_t = bp.tile([P, N], mybir.dt.uint8, tag="pred")
                pb = pred[b]
                nc.sync.dma_start(
                    out=pred_t,
                    in_=bass.AP(tensor=pb.tensor, offset=pb.offset,
                                ap=[[0, P]] + pb.ap))

                # ---- x^T (c on partitions) via PE transpose ----
                xT_t = bp.tile([P, CC, N], F32R, tag="xT")
                for cc in range(CC):
                    ps = ps_proj.tile([P, N], F32, tag="proj")
                    for it in range(IT):
                        nc.tensor.transpose(
                            ps[:, it * P:(it + 1) * P],
                            x_t[:, it, cc * P:(cc + 1) * P], ident)
                    nc.scalar.activation(
                        xT_t[:, cc, :], ps, mybir.ActivationFunctionType.Copy)

                # ---- mean(v) for fully-masked queries (host-computed) ----
                vmean_t = bp.tile([P, ET], F32, tag="vmean")
                nc.sync.dma_start(
                    out=vmean_t, in_=vmt[b].rearrange("(t p) -> p t", p=P))

                # ---- projections q^T, k^T (e on partitions) ----
                qT_t = bp.tile([P, ET, N], F32R, tag="qT")
                kT_t = bp.tile([P, ET, N], F32R, tag="kT")
                for w_t, dst in ((wq_t, qT_t), (wk_t, kT_t)):
                    for ec in range(ET):
                        ps = ps_proj.tile([P, N], F32, tag="proj")
                        for cc in range(CC):
                            nc.tensor.matmul(
                                ps, w_t[:, cc, ec * P:(ec + 1) * P],
                                xT_t[:, cc, :],
                                start=(cc == 0), stop=(cc == CC - 1))
                        nc.scalar.activation(
                            dst[:, ec, :], ps,
                            mybir.ActivationFunctionType.Copy)

                # ---- v (seq on partitions) in bf16, with ones column ----
                v_t = bp.tile([P, IT, H, DH + 1], BF16, tag="v")
                nc.vector.memset(v_t, 1.0)
                for jt in range(IT):
                    ps = ps_proj.tile([P, N], F32, tag="proj")
                    for cc in range(CC):
                        nc.tensor.matmul(
                            ps, xT_t[:, cc, jt * P:(jt + 1) * P],
                            wv_t[:, cc, :],
                            start=(cc == 0), stop=(cc == CC - 1))
                    nc.scalar.activation(
                        v_t[:, jt, :, 0:DH], ps,
                        mybir.ActivationFunctionType.Copy)

                # ---- gates^T (e on partitions) with bias ----
                gT_t = bp.tile([P, ET, N], F32, tag="gT")
                for ec in range(ET):
                    ps = ps_proj.tile([P, N], F32, tag="proj")
                    for cc in range(CC):
                        nc.tensor.matmul(
                            ps, wg_t[:, cc, ec * P:(ec + 1) * P],
                            xT_t[:, cc, :],
                            start=(cc == 0), stop=(cc == CC - 1))
                    nc.vector.tensor_scalar_add(
                        gT_t[:, ec, :], in0=ps, scalar1=bg_t[:, ec:ec + 1])

                # ---- attention heads ----
                og_t = bp.tile([P, ET, N], F32, tag="og")
                for h in range(H):
                    po = (h % 2) * DH
                    ec = h // 2
                    p_t = hp.tile([P, IT, N], BF16, tag="p")
                    for jt in range(IT):
                        s_ps = ps_sp.tile([P, N], F32, tag="s")
                        nc.tensor.matmul(
                            s_ps,
                            kT_t[po:po + DH, ec, jt * P:(jt + 1) * P],
                            qT_t[po:po + DH, ec, :],
                            start=True, stop=True)
                        nc.scalar.activation(
                            p_t[:, jt, :], s_ps,
                            mybir.ActivationFunctionType.Exp,
                            bias=mjb_t[:, jt:jt + 1], scale=1.0)
                        eng = nc.vector if jt % 2 == 0 else nc.gpsimd
                        eng.tensor_mul(
                            p_t[:, jt, :], p_t[:, jt, :], expb_t[:, h, jt, :])
                    ot_ps = ps_otp.tile([P, N], F32, tag="ot")
                    for jt in range(IT):
                        nc.tensor.matmul(
                            ot_ps[0:DH + 1, :], v_t[:, jt, h, :], p_t[:, jt, :],
                            start=(jt == 0), stop=(jt == IT - 1))
                    recip_t = hp.tile([1, N], F32R, tag="recip")
                    with nc.allow_low_precision(reason="fp32r recip for PE broadcast"):
                        nc.vector.reciprocal(recip_t, ot_ps[DH:DH + 1, :])
                    rb_ps = ps_rbp.tile([DH, N], F32, tag="rb")
                    nc.tensor.matmul(rb_ps, ones1, recip_t,
                                     start=True, stop=True)
                    rb_t = hp.tile([DH, N], F32, tag="rbs")
                    nc.scalar.activation(
                        rb_t, rb_ps, mybir.ActivationFunctionType.Copy)
                    nc.vector.tensor_mul(
                        og_t[po:po + DH, ec, :], ot_ps[0:DH, :], rb_t)

                # ---- fully-masked queries: overwrite with mean(v) ----
                nc.vector.copy_predicated(
                    og_t,
                    bass.AP(tensor=pred_t.tensor, offset=pred_t.offset,
                            ap=[pred_t.ap[0], [0, ET], pred_t.ap[1]]),
                    bass.AP(tensor=vmean_t.tensor, offset=vmean_t.offset,
                            ap=[vmean_t.ap[0], vmean_t.ap[1], [0, N]]))

                # ---- gating (sbuf-only, keep it off the vector engine) ----
                pg_t = bp.tile([P, ET, N], F32R, tag="pg")
                nc.gpsimd.tensor_mul(pg_t, og_t, gT_t)

                # ---- output projection ----
                y_t = bp.tile([P, IT, DIM], F32, tag="y")
                for it in range(IT):
                    y_ps = ps_proj.tile([P, DIM], F32, tag="proj")
                    for ec in range(ET):
                        nc.tensor.matmul(
                            y_ps, pg_t[:, ec, it * P:(it + 1) * P],
                            wo_t[:, ec, :],
                            start=(ec == 0), stop=(ec == ET - 1))
                    nc.vector.tensor_add(y_t[:, it, :], in0=y_ps, in1=bo_t)
                for _it in range(IT):
                    nc.sync.dma_start(out=out[b, _it * P:(_it + 1) * P, :],
                                      in_=y_t[:, _it, :])

    nc.compile()
    return nc


_NC_CACHE = {}


def kernel(x, mask, attn_bias, Wq, Wkv, Wo, bo, Wg, bg):
    x = np.asarray(x, dtype=np.float32)
    mask = np.asarray(mask)
    attn_bias = np.asarray(attn_bias, dtype=np.float32)
    Wq = np.asarray(Wq, dtype=np.float32)
    Wkv = np.asarray(Wkv, dtype=np.float32)
    Wo = np.asarray(Wo, dtype=np.float32)
    bo = np.asarray(bo, dtype=np.float32)
    Wg = np.asarray(Wg, dtype=np.float32)
    bg = np.asarray(bg, dtype=np.float32)

    wq_s = (Wq * SCALE).astype(np.float32)
    wk_s = np.ascontiguousarray(Wkv[:, :INNER])
    wv_s = np.ascontiguousarray(Wkv[:, INNER:])
    expb = np.ascontiguousarray(
        np.exp(attn_bias[0]).transpose(0, 2, 1)).astype(ml_dtypes.bfloat16)
    mjb = np.where(mask, 0.0, NEG).astype(np.float32)
    vmt_full = (x.mean(axis=1) @ wv_s).astype(np.float32)  # [B, INNER]
    pred = np.where(mask, 0, 1).astype(np.uint8)

    if "nc" not in _NC_CACHE:
        _NC_CACHE["nc"] = build_kernel()
    nc = _NC_CACHE["nc"]

    in_maps = []
    for c in range(NCORES):
        sl = slice(c * BPC, (c + 1) * BPC)
        in_maps.append({
            "x": np.ascontiguousarray(x[sl]),
            "mjb": np.ascontiguousarray(mjb[sl]),
            "pred": np.ascontiguousarray(pred[sl]),
            "expb": expb,
            "vmt": np.ascontiguousarray(vmt_full[sl]),
            "onesd": np.ones((1, DH), dtype=np.float32),
            "wq": wq_s, "wk": wk_s, "wv": wv_s, "wg": Wg,
            "wo": Wo, "bg": bg, "bo": bo,
        })
    res = run_bass_kernel_spmd(nc, in_maps, core_ids=list(range(NCORES)))
    outs = [np.asarray(r["out"]) for r in res.results]
    return np.concatenate(outs, axis=0).astype(np.float32)
